# revision 58
# baseline (speedup 1.0000x reference)
"""Trainium2 Bass kernel for ExtensibleAttention (sparse_attention).

Strategy: data-parallel over the 65536 tokens (N*L flattened) across 8
NeuronCores; the small 256-dim projection weights are replicated. All
per-token math is fused into one pass per 512-token tile.

The q/k/pos/v/Wo1 projections run as fp8e4m3 DoubleRow matmuls (2
contraction slices per instruction at 0.5 cycles/row = 4x fp32r rate).
Accuracy is recovered with a 3-term split: activations are shipped as an
fp8 (hi, lo) pair, weights are pre-scaled by 16 on the host (so their
fp8 residual stays out of e4m3's subnormal range) and also split, and
the kernel accumulates Wh*xh + Wh*xl + Wl*xh in PSUM -> ~8e-4 rms error
per matmul, better than bf16. The x16 weight scale is folded downstream
for free: the q*k product carries 256x which the softmax exp() absorbs
in its scale argument, Wo2 is host-divided by 16 to absorb the hidden
scale, and the bmat head-broadcast matrix carries 1/16 to unscale v.

The q*k product reads the q/k PSUM banks directly on DVE (no ACT
copies); v's PSUM->SBUF copies run on the otherwise idle GPSIMD. Head
reductions, the k-broadcast of qk, the sum over K, and the
head->channel broadcast of wv are matmuls against small constant
matrices. The Wo2 columns are host-permuted from (h,k,c) to (c,h,k)
order so the x/y coordinates occupy partition halves, making the
grid-sample weight product a single partition-offset vector multiply.
"""

import numpy as np
from contextlib import ExitStack

import ml_dtypes

import concourse.bacc as bacc
import concourse.tile as tile
from concourse import mybir

F32 = mybir.dt.float32
F32R = mybir.dt.float32r
F8 = mybir.dt.float8e4
AF = mybir.ActivationFunctionType
DR = mybir.MatmulPerfMode.DoubleRow
E4 = ml_dtypes.float8_e4m3

N, L, C, H, KP, D = 4, 16384, 256, 8, 4, 32
NCORES = 8
TOKS = N * L // NCORES  # 8192 tokens per core
TLOAD = 512             # tokens per DMA load tile
TCOMP = 512             # tokens per compute tile (PSUM free-dim limit, fp32)
ALPHA = 16.0            # host pre-scale on fp8-split weights
SIGMA = float(1.0 / np.sqrt(D))


def _build(toks=TOKS, tload=TLOAD, with_bias=False, psa=3, psb=5,
           hid_dve=True, order=0, v_late=False, prefetch=1,
           qkp2=False, m8=True, t2_act=False, ew_dve=True,
           last_split=0, dma_pool=False, early_load=True,
           out_pool=False, tail_split=1, spread0=True,
           store_act=False, spread1=False, fuse_s=False, kc_dve=False,
           ksb_bufs=1, hid_bufs=2, m_bufs=2, v_bufs=3, vlast=0,
           inp_bufs=4, psh=0, w_spread=True, split_store=True,
           hflast=1, drl=0, kfirst=False):
    nc = bacc.Bacc(trn_type="TRN2")
    dram = {}

    def din(name, shape, dt=None):
        dram[name] = nc.dram_tensor(name, list(shape), dt or F32R,
                                    kind="ExternalInput")
        return dram[name]

    xq = din("xq", (128, 2, 2, toks), F8)
    xk = din("xk", (128, 2, 2, toks), F8)
    xv = din("xv", (128, 2, 2, toks), F8)
    xp = din("xp", (128, 2, 2, toks), F8)
    ref = din("ref", (2, toks))
    din("wq", (128, 2, 2, 256), F8)
    din("wk", (128, 2, 2, 256), F8)
    din("wv", (128, 2, 2, 256), F8)
    din("wp", (128, 2, 2, 256), F8)
    din("wo1", (128, 2, 2, 512), F8)
    din("wo2", (128, 4, 64))
    din("wo", (128, 2, 256))
    din("bo1", (128, 4))
    din("bwof", (64, 1))
    din("smat", (64, 32))
    din("amat", (128, 2, 32), F8)
    din("cmat", (32, 8))
    din("cmat2", (64, 16))
    din("bmat", (8, 256))
    din("pmat", (2, 64))
    if with_bias:
        din("ones", (1, 512))
        din("bqp", (1, 256))
        din("bkp", (1, 256))
        din("bvr", (1, 256))
        din("bor", (1, 256))
    out = nc.dram_tensor("out", [128, 2, toks], F32, kind="ExternalOutput")

    nload = toks // tload
    nsub = tload // TCOMP
    T = TCOMP

    with tile.TileContext(nc) as tc, ExitStack() as ctx:
        singles = ctx.enter_context(tc.tile_pool(name="singles", bufs=1))
        inp = ctx.enter_context(tc.tile_pool(name="inp", bufs=inp_bufs))
        work = ctx.enter_context(tc.tile_pool(name="work", bufs=2))
        psA = ctx.enter_context(tc.tile_pool(name="psA", bufs=psa, space="PSUM"))
        psB = ctx.enter_context(tc.tile_pool(name="psB", bufs=psb - psh,
                                             space="PSUM"))
        psH = (ctx.enter_context(tc.tile_pool(name="psH", bufs=psh,
                                              space="PSUM"))
               if psh else psB)

        _wq = [nc.sync, nc.scalar, nc.gpsimd]
        _wqi = [0]

        def load1(name, shape, dt=F32R):
            t = singles.tile(list(shape), dt, name=f"sb_{name}")
            eng = _wq[_wqi[0] % 3] if w_spread else nc.sync
            _wqi[0] += 1
            eng.dma_start(out=t, in_=dram[name][:])
            return t

        mm = nc.tensor.matmul

        def mm3(ps, w_s, x_t, s, mc, start, stop, lo_first=False, n=3):
            """fp8 DoubleRow accumulation: Wh*xh + Wh*xl (+ Wl*xh for n=3).
            lo_first emits the Wl term first so a preceding group ending on
            the same Wl stationary elides one LD_WEIGHTS."""
            m128 = slice(mc * 128, (mc + 1) * 128)
            terms = [(0, 0), (0, 1), (1, 0)][:n]
            if lo_first and n == 3:
                terms = [(1, 0), (0, 0), (0, 1)]
            for i, (wi, xi) in enumerate(terms):
                mm(ps, w_s[:, :, wi, m128], x_t[:, :, xi, s],
                   start=start and i == 0, stop=stop and i == len(terms) - 1,
                   perf_mode=DR)

        def load_tile(lt, spread=False):
            eng = nc.gpsimd if dma_pool else nc.sync
            e2 = nc.scalar if spread else eng
            e3 = nc.gpsimd if spread else eng
            t0 = lt * tload
            xv_t = inp.tile([128, 2, 2, tload], F8, tag="xv")
            eng.dma_start(out=xv_t, in_=xv[:, :, :, t0:t0 + tload])
            xq_t = inp.tile([128, 2, 2, tload], F8, tag="xq")
            e2.dma_start(out=xq_t, in_=xq[:, :, :, t0:t0 + tload])
            xp_t = inp.tile([128, 2, 2, tload], F8, tag="xp")
            e3.dma_start(out=xp_t, in_=xp[:, :, :, t0:t0 + tload])
            xk_t = inp.tile([128, 2, 2, tload], F8, tag="xk")
            eng.dma_start(out=xk_t, in_=xk[:, :, :, t0:t0 + tload])
            ref_t = inp.tile([2, tload], F32R, tag="ref")
            eng.dma_start(out=ref_t, in_=ref[:, t0:t0 + tload])
            return xq_t, xp_t, xk_t, xv_t, ref_t

        def emit_v(ld, lo, tz):
            xq_t, xp_t, xk_t, xv_t, ref_t = ld
            s = slice(lo, lo + tz)
            v_sb = work.tile([128, 2, tz], F32, tag="v", bufs=v_bufs)
            for mc in range(2):
                m128 = slice(mc * 128, (mc + 1) * 128)
                v_ps = psA.tile([128, tz], F32, tag="bigA")
                mm3(v_ps, wv_s, xv_t, s, mc, True, not with_bias)
                if with_bias:
                    mm(v_ps, bvr_s[:, m128], ones_s[:, :tz], start=False,
                       stop=True)
                nc.scalar.copy(v_sb[:, mc, :], v_ps)
            return v_sb

        def emit_hid_off(ld, lo, tz):
            xq_t, xp_t, xk_t, xv_t, ref_t = ld
            s = slice(lo, lo + tz)
            hid_sb = work.tile([128, 4, tz], F32R, tag="hid", bufs=hid_bufs)
            for j in range(4):
                h_ps = psH.tile([128, tz], F32,
                                tag="hps" if psh else "small")
                j128 = slice(j * 128, (j + 1) * 128)
                mm(h_ps, wo1_s[:, :, 0, j128], xq_t[:, :, 0, s], start=True,
                   stop=False, perf_mode=DR)
                mm(h_ps, wo1_s[:, :, 0, j128], xq_t[:, :, 1, s], start=False,
                   stop=False, perf_mode=DR)
                mm(h_ps, wo1_s[:, :, 1, j128], xq_t[:, :, 0, s], start=False,
                   stop=True, perf_mode=DR)
                if j == 3 and not with_bias and hid_dve:
                    nc.vector.tensor_scalar_max(hid_sb[:, j, :], h_ps, 0.0)
                else:
                    nc.scalar.activation(hid_sb[:, j, :], h_ps, AF.Relu,
                                         bias=bo1_s[:, j:j + 1], scale=1.0)
            off_ps = psB.tile([64, tz], F32, tag="small")
            for j in range(4):
                mm(off_ps, wo2_s[:, j, :], hid_sb[:, j, :],
                   start=(j == 0), stop=False)
            mm(off_ps, pmat_s, ref_t[:, s], start=False, stop=True)
            return off_ps

        def stage1(ld, lo, tz, vlate=None, hid_first=False):
            """Projection matmuls + q*k product + hidden/offset MLP."""
            vlate = v_late if vlate is None else vlate
            xq_t, xp_t, xk_t, xv_t, ref_t = ld
            s = slice(lo, lo + tz)
            if hid_first:
                off_ps = emit_hid_off(ld, lo, tz)
            if not vlate and not hid_first:
                v_sb = emit_v(ld, lo, tz)

            # q/k projections (+pos fused into the PSUM accumulation); k is
            # copied to SBUF on ACT (DVE may read only one PSUM operand) and
            # the q*k product reads the q PSUM bank directly on DVE
            m_sb = work.tile([128, 2, tz], F8 if m8 else F32R, tag="m",
                             bufs=m_bufs)
            k_sb = work.tile([128, 2, tz], F32, tag="ksb", bufs=ksb_bufs)
            for mc in range(2):
                m128 = slice(mc * 128, (mc + 1) * 128)
                nq = 2 if qkp2 else 3
                if kfirst:
                    k_ps = psA.tile([128, tz], F32, tag="bigA")
                    mm3(k_ps, wk_s, xk_t, s, mc, True, False, n=nq)
                    mm3(k_ps, wp_s, xp_t, s, mc, False, not with_bias, n=nq)
                    if with_bias:
                        mm(k_ps, bkp_s[:, m128], ones_s[:, :tz], start=False,
                           stop=True)
                    q_ps = psA.tile([128, tz], F32, tag="bigA")
                    mm3(q_ps, wp_s, xp_t, s, mc, True, False, lo_first=True,
                        n=nq)
                    mm3(q_ps, wq_s, xq_t, s, mc, False, not with_bias, n=nq)
                    if with_bias:
                        mm(q_ps, bqp_s[:, m128], ones_s[:, :tz], start=False,
                           stop=True)
                else:
                    q_ps = psA.tile([128, tz], F32, tag="bigA")
                    mm3(q_ps, wq_s, xq_t, s, mc, True, False, n=nq)
                    mm3(q_ps, wp_s, xp_t, s, mc, False, not with_bias, n=nq)
                    if with_bias:
                        mm(q_ps, bqp_s[:, m128], ones_s[:, :tz], start=False,
                           stop=True)
                    k_ps = psA.tile([128, tz], F32, tag="bigA")
                    # Wpos first (lo term leading for 3-term) so q's trailing
                    # Wpos stationary carries over without a reload
                    mm3(k_ps, wp_s, xp_t, s, mc, True, False, lo_first=True,
                        n=nq)
                    mm3(k_ps, wk_s, xk_t, s, mc, False, not with_bias, n=nq)
                    if with_bias:
                        mm(k_ps, bkp_s[:, m128], ones_s[:, :tz], start=False,
                           stop=True)
                if kc_dve:
                    nc.vector.tensor_copy(k_sb[:, mc, :], k_ps)
                else:
                    nc.scalar.copy(k_sb[:, mc, :], k_ps)
                if m8:
                    # m = (q/256)*k in fp8 -- unscales the 16x on q and k so
                    # fp8's +-448 range holds the products
                    nc.vector.scalar_tensor_tensor(
                        m_sb[:, mc, :], q_ps, 1.0 / 256.0, k_sb[:, mc, :],
                        op0=mybir.AluOpType.mult, op1=mybir.AluOpType.mult)
                else:
                    nc.vector.tensor_mul(m_sb[:, mc, :], q_ps, k_sb[:, mc, :])
            if vlate:
                v_sb = emit_v(ld, lo, tz)
            if hid_first:
                v_sb = emit_v(ld, lo, tz)
            else:
                if vlate:
                    v_sb = emit_v(ld, lo, tz)
                off_ps = emit_hid_off(ld, lo, tz)
            return m_sb, v_sb, off_ps, tz

        def stage2a(state, o0=0, osz=None, drain=False):
            """Head-sum of q*k, grid-sample weight w, softmax partial sums.
            o0/osz optionally restrict to a token sub-range of the tile
            (used to pipeline the final tile's drain)."""
            m_sb, v_sb, off_ps, tz = state
            if osz is None:
                osz = tz
            so = slice(o0, o0 + osz)
            tz = osz

            qk_ps = psB.tile([32, tz], F32, tag="small")
            if m8:
                mm(qk_ps, amat_s, m_sb[:, :, so], start=True, stop=True,
                   perf_mode=DR)
            else:
                mm(qk_ps, amat_s[:, 0:32], m_sb[:, 0, so], start=True,
                   stop=False)
                mm(qk_ps, amat_s[:, 32:64], m_sb[:, 1, so], start=False,
                   stop=True)

            # w = relu(1-|sp_x-.5|)*relu(1-|sp_y-.5|); computed sign-flipped
            # as t2n = min(t1,1)-1 = -relu(1-t1) on GPSIMD (one tensor_scalar)
            # -- the two negative factors cancel in the product. The y half
            # is moved to partitions 0-31 with a PE row-select matmul since
            # DVE can't pair operands at different base partitions.
            t1_sb = work.tile([64, tz], F32, tag="t1")
            nc.scalar.activation(t1_sb, off_ps[:, so], AF.Abs, bias=bwof_s,
                                 scale=1.0)
            t2_sb = work.tile([64, tz], F32R, tag="t2")
            if t2_act:
                nc.scalar.activation(t2_sb, t1_sb, AF.Relu, bias=1.0,
                                     scale=-1.0)
            else:
                nc.gpsimd.tensor_scalar(t2_sb, t1_sb, 1.0, 1.0,
                                        mybir.AluOpType.min,
                                        mybir.AluOpType.subtract)
            t2y_ps = psB.tile([32, tz], F32, tag="small")
            mm(t2y_ps, smat_s, t2_sb, start=True, stop=True)
            w_sb = work.tile([32, tz], F32, tag="w")
            nc.vector.tensor_mul(w_sb, t2_sb[0:32, :], t2y_ps)

            # softmax over K: e = exp(qk*w/(256*sqrt(D))) -- the 1/256
            # unscales the 16x on each of q and k; the qk head-sum is read
            # straight from PSUM; e*w runs on the otherwise idle GPSIMD
            lg_sb = work.tile([32, tz], F32, tag="lg")
            nc.vector.tensor_mul(lg_sb, qk_ps, w_sb)
            escale = SIGMA if m8 else SIGMA / (ALPHA * ALPHA)
            if fuse_s:
                # e and e*w stacked in one [64,T] tile -> ONE [64->16]
                # matmul yields both softmax partial sums
                eew_sb = work.tile([64, tz], F32R, tag="eew")
                nc.scalar.activation(eew_sb[0:32, :], lg_sb, AF.Exp,
                                     bias=0.0, scale=escale)
                nc.vector.tensor_mul(eew_sb[32:64, :], eew_sb[0:32, :], w_sb)
                s12_ps = psB.tile([16, tz], F32, tag="small")
                mm(s12_ps, cmat2_s, eew_sb, start=True, stop=True)
                s1_ps, s2_ps = s12_ps[0:8, :], s12_ps[8:16, :]
            else:
                e_sb = work.tile([32, tz], F32R, tag="e")
                nc.scalar.activation(e_sb, lg_sb, AF.Exp, bias=0.0,
                                     scale=escale)
                # s1 fires immediately after exp so recip overlaps the e*w
                # product feeding s2
                s1_ps = psB.tile([8, tz], F32, tag="small")
                mm(s1_ps, cmat_s, e_sb, start=True, stop=True)
                ew_sb = work.tile([32, tz], F32R, tag="ew")
                if ew_dve and not drain:
                    nc.vector.tensor_mul(ew_sb, e_sb, w_sb)
                else:
                    nc.gpsimd.tensor_mul(ew_sb, e_sb, w_sb)
                s2_ps = psB.tile([8, tz], F32, tag="small")
                mm(s2_ps, cmat_s, ew_sb, start=True, stop=True)
            return s1_ps, s2_ps, v_sb, tz, o0

        def stage2b(state, g0, drain=False):
            """Softmax normalization, ov = v*wv, out-projection, store."""
            s1_ps, s2_ps, v_sb, tz, o0 = state
            so = slice(o0, o0 + tz)
            r1_sb = work.tile([8, tz], F32, tag="r1")
            nc.vector.reciprocal(r1_sb, s1_ps)
            wv_sb = work.tile([8, tz], F32R, tag="wvv")
            nc.vector.tensor_mul(wv_sb, s2_ps, r1_sb)

            # ov = v * wv (broadcast head->channels via matmul; bmat carries
            # 1/16 to unscale the 16x on v)
            ov_sb = work.tile([128, 2, tz], F32R, tag="ov")
            for mc in range(2):
                wvx_ps = psB.tile([128, tz], F32, tag="small")
                mm(wvx_ps, bmat_s[:, mc * 128:(mc + 1) * 128], wv_sb,
                   start=True, stop=True)
                nc.vector.tensor_mul(ov_sb[:, mc, :], v_sb[:, mc, so],
                                     wvx_ps)

            # outT = Wout.T @ ov, channel-major [256, T] (stationary = Wout
            # chunks: 4 LD_WEIGHTS instead of 8); host transposes back
            o_sb = work.tile([128, 2, tz], F32, tag="osb")
            for oc in range(2):
                o_ps = psB.tile([128, tz], F32, tag="small")
                oc128 = slice(oc * 128, (oc + 1) * 128)
                mm(o_ps, wo_s[:, 0, oc128], ov_sb[:, 0, :], start=True,
                   stop=False)
                mm(o_ps, wo_s[:, 1, oc128], ov_sb[:, 1, :], start=False,
                   stop=not with_bias)
                if with_bias:
                    mm(o_ps, bor_s[0:1, oc128], ones_s[0:1, :tz],
                       start=False, stop=True)
                if oc == 0 and not drain:
                    nc.vector.tensor_copy(o_sb[:, oc, :], o_ps)
                else:
                    nc.scalar.copy(o_sb[:, oc, :], o_ps)
            oeng = (nc.gpsimd if out_pool
                    else nc.scalar if store_act else nc.sync)
            if split_store:
                for oc in range(2):
                    oeng.dma_start(out=out[:, oc, g0 + o0:g0 + o0 + tz],
                                   in_=o_sb[:, oc, :])
            else:
                oeng.dma_start(out=out[:, :, g0 + o0:g0 + o0 + tz], in_=o_sb)

        # 3-deep software pipeline: per iteration emit tile i's matmul-heavy
        # stage1, then tile i-2's output tail (stage2b), then tile i-1's
        # softmax chain (stage2a) — PE stays dense while ACT/DVE chains of
        # earlier tiles drain. stage2b(i-2) must precede stage2a(i-1) so the
        # s1/s2 PSUM slots recycle in trace order.
        assert nsub in (1, 2)
        units = []
        for lt in range(nload):
            for lo in range(0, tload, TCOMP):
                if lt >= nload - last_split:
                    units.append((lt, lo, TCOMP // 2))
                    units.append((lt, lo + TCOMP // 2, TCOMP // 2))
                else:
                    units.append((lt, lo, TCOMP))
        p1 = p2 = None  # (state, g0) for stage2a / stage2b
        # first input tile before the weights so the PE can start ASAP;
        # weights ordered by first use
        wv_s = load1("wv", (128, 2, 2, 256), F8)
        ld = load_tile(0, spread=spread0)
        wq_s = load1("wq", (128, 2, 2, 256), F8)
        wp_s = load1("wp", (128, 2, 2, 256), F8)
        wk_s = load1("wk", (128, 2, 2, 256), F8)
        if early_load and nload > 1:
            if spread1:
                _sv = nc.sync
                nc.sync = nc.gpsimd
                ld1 = load_tile(1)
                nc.sync = _sv
            else:
                ld1 = load_tile(1)
        else:
            ld1 = None
        wo1_s = load1("wo1", (128, 2, 2, 512), F8)
        bo1_s = load1("bo1", (128, 4))
        amat_s = load1("amat", (128, 2, 32), F8)
        wo2_s = load1("wo2", (128, 4, 64))
        pmat_s = load1("pmat", (2, 64))
        bwof_s = load1("bwof", (64, 1))
        smat_s = load1("smat", (64, 32))
        cmat_s = load1("cmat", (32, 8))
        cmat2_s = load1("cmat2", (64, 16)) if fuse_s else None
        bmat_s = load1("bmat", (8, 256))
        wo_s = load1("wo", (128, 2, 256))
        if with_bias:
            bqp_s = load1("bqp", (1, 256))
            bkp_s = load1("bkp", (1, 256))
            bvr_s = load1("bvr", (1, 256))
            bor_s = load1("bor", (1, 256))
            ones_s = load1("ones", (1, 512))
        lds = {0: ld}
        if early_load and nload > 1:
            lds[1] = ld1
        for pf in range(1, min(prefetch, nload)):
            if pf not in lds:
                lds[pf] = load_tile(pf)
        for ui, (lt, lo, tz) in enumerate(units):
            nxt = ui + prefetch
            if nxt < len(units) and units[nxt][0] not in lds:
                lds[units[nxt][0]] = load_tile(units[nxt][0])
            ld = lds[lt]
            vl = True if (len(units) - ui) <= vlast else None
            hf = (len(units) - ui) <= hflast
            if order == 0:
                state = stage1(ld, lo, tz, vlate=vl, hid_first=hf)
                if p2 is not None:
                    stage2b(*p2)
                    p2 = None
                if p1 is not None:
                    st2, g0p = p1
                    p2 = (stage2a(st2), g0p)
            elif order == 1:
                if p2 is not None:
                    stage2b(*p2)
                    p2 = None
                state = stage1(ld, lo, tz)
                if p1 is not None:
                    st2, g0p = p1
                    p2 = (stage2a(st2), g0p)
            elif order == 2:
                if p2 is not None:
                    stage2b(*p2)
                    p2 = None
                if p1 is not None:
                    st2, g0p = p1
                    p2 = (stage2a(st2), g0p)
                state = stage1(ld, lo, tz)
            else:  # order == 3: 2-deep, 2a(i) right in iteration i
                state = stage1(ld, lo, tz)
                if p2 is not None:
                    stage2b(*p2)
                    p2 = None
                p2 = (stage2a((state, )[0]), lt * tload + lo)
            p1 = (state, lt * tload + lo)
        if p2 is not None:
            stage2b(*p2)
        if order != 3:
            st2, g0p = p1
            ftz = st2[3]
            nchunk = max(1, tail_split)
            csz = ftz // nchunk
            dr = drl > 0
            parts = [stage2a(st2, o0=ci * csz, osz=csz, drain=dr)
                     for ci in range(nchunk)]
            for part in parts:
                stage2b(part, g0p, drain=dr)

    nc.compile()
    return nc


def _consts():
    amat = np.zeros((128, 64), np.float32)
    for mc in range(2):
        for d in range(128):
            h = mc * 4 + d // 32
            for k in range(KP):
                amat[d, mc * 32 + h * KP + k] = 1.0
    cmat = np.zeros((32, 8), np.float32)
    for j in range(32):
        cmat[j, j // KP] = 1.0
    cmat2 = np.zeros((64, 16), np.float32)
    for j in range(32):
        cmat2[j, j // KP] = 1.0
        cmat2[32 + j, 8 + j // KP] = 1.0
    bmat = np.zeros((8, 256), np.float32)
    for mc in range(2):
        for c in range(128):
            bmat[mc * 4 + c // 32, mc * 128 + c] = 1.0 / ALPHA
    pmat = np.zeros((2, 64), np.float32)
    for r in range(64):
        pmat[r // 32, r] = 1.0
    smat = np.zeros((64, 32), np.float32)
    for j in range(32):
        smat[32 + j, j] = 1.0
    return amat, cmat, cmat2, bmat, pmat, smat


def _wsplit(w):
    # [256, O] -> [128, 2, O]  (row kc*128+p  ->  [p, kc, :])
    o = w.shape[1]
    return np.ascontiguousarray(w.reshape(2, 128, o).transpose(1, 0, 2))


def _xsplit(x):
    # [T, 256] token-major -> [128, 2, T] channel-major chunks
    t = x.shape[0]
    return np.ascontiguousarray(x.T.reshape(2, 128, t).transpose(1, 0, 2))


def _split8(a):
    # [128, 2, O] f32 -> [128, 2, 2, O] fp8 (dim2 = hi, lo of the value)
    hi = a.astype(E4)
    lo = (a - hi.astype(np.float32)).astype(E4)
    return np.ascontiguousarray(np.stack([hi, lo], axis=2))


def _host_maps(inputs, toks, ncores):
    f32 = lambda v: np.asarray(v, dtype=np.float32)
    query = f32(inputs["query"]).reshape(-1, C)
    key = f32(inputs["key"]).reshape(-1, C)
    value = f32(inputs["value"]).reshape(-1, C)
    pos = f32(inputs["pos_embed"]).reshape(-1, C)
    refp = f32(inputs["reference_points"]).reshape(-1, 2)

    # permute Wo2 columns (h,k,c) -> (c,h,k)
    perm = [h * (KP * 2) + k * 2 + c for c in range(2) for h in range(H)
            for k in range(KP)]
    wo2p = f32(inputs["Wo2"])[:, perm]
    bo2p = f32(inputs["bo2"])[perm]

    amat, cmat, cmat2, bmat, pmat, smat = _consts()
    bqp = f32(inputs["bq"]) + f32(inputs["bpos"])
    bkp = f32(inputs["bk"]) + f32(inputs["bpos"])
    bv = f32(inputs["bv"])
    bout = f32(inputs["bout"])
    with_bias = any(np.any(b != 0) for b in (bqp, bkp, bv, bout))

    # Wo2 operand (hidden) carries 16x -> divide Wo2 by 16 to compensate
    wo2r = np.ascontiguousarray(
        (wo2p / ALPHA).reshape(4, 128, 64).transpose(1, 0, 2))
    shared = {
        "wq": _split8(_wsplit(ALPHA * f32(inputs["Wq"]))),
        "wk": _split8(_wsplit(ALPHA * f32(inputs["Wk"]))),
        "wv": _split8(_wsplit(ALPHA * f32(inputs["Wv"]))),
        "wp": _split8(_wsplit(ALPHA * f32(inputs["Wpos"]))),
        "wo1": _split8(_wsplit(ALPHA * f32(inputs["Wo1"]))),
        "wo2": wo2r,
        "wo": _wsplit(f32(inputs["Wout"])),
        "bo1": np.ascontiguousarray(
            (ALPHA * f32(inputs["bo1"])).reshape(4, 128).T),
        "bwof": np.ascontiguousarray((bo2p - 0.5).reshape(64, 1)),
        "smat": smat,
        "amat": np.ascontiguousarray(amat.reshape(128, 2, 32)).astype(E4),
        "cmat": cmat, "cmat2": cmat2, "bmat": bmat, "pmat": pmat,
    }
    if with_bias:
        shared["ones"] = np.ones((1, 512), np.float32)
        shared["bqp"] = ALPHA * bqp.reshape(1, 256)
        shared["bkp"] = ALPHA * bkp.reshape(1, 256)
        shared["bvr"] = ALPHA * bv.reshape(1, 256)
        shared["bor"] = bout.reshape(1, 256)

    in_maps = []
    for cid in range(ncores):
        sl = slice(cid * toks, (cid + 1) * toks)
        m = dict(shared)
        m["xq"] = _split8(_xsplit(query[sl]))
        m["xk"] = _split8(_xsplit(key[sl]))
        m["xv"] = _split8(_xsplit(value[sl]))
        m["xp"] = _split8(_xsplit(pos[sl]))
        m["ref"] = np.ascontiguousarray(refp[sl].T)
        in_maps.append(m)
    return in_maps, with_bias


_NC_CACHE = {}


def kernel(**inputs):
    from concourse.bass_utils import run_bass_kernel_spmd

    in_maps, with_bias = _host_maps(inputs, TOKS, NCORES)
    ck = ("full", with_bias)
    if ck not in _NC_CACHE:
        _NC_CACHE[ck] = _build(toks=TOKS, tload=TLOAD, with_bias=with_bias)
    nc = _NC_CACHE[ck]
    res = run_bass_kernel_spmd(nc, in_maps, core_ids=list(range(NCORES)))
    # out is channel-major [128, 2, toks]: out[t, kc*128+p] = o[p, kc, t]
    outs = [np.asarray(r["out"]).transpose(2, 1, 0).reshape(TOKS, C)
            for r in res.results]
    full = np.concatenate(outs, axis=0).reshape(N, L, C)
    return np.ascontiguousarray(full.astype(np.float32))


# revision 61
# speedup vs baseline: 1.0005x; 1.0005x over previous
"""Trainium2 Bass kernel for ExtensibleAttention (sparse_attention).

Strategy: data-parallel over the 65536 tokens (N*L flattened) across 8
NeuronCores; the small 256-dim projection weights are replicated. All
per-token math is fused into one pass per 512-token tile.

The q/k/pos/v/Wo1 projections run as fp8e4m3 DoubleRow matmuls (2
contraction slices per instruction at 0.5 cycles/row = 4x fp32r rate).
Accuracy is recovered with a 3-term split: activations are shipped as an
fp8 (hi, lo) pair, weights are pre-scaled by 16 on the host (so their
fp8 residual stays out of e4m3's subnormal range) and also split, and
the kernel accumulates Wh*xh + Wh*xl + Wl*xh in PSUM -> ~8e-4 rms error
per matmul, better than bf16. The x16 weight scale is folded downstream
for free: the q*k product carries 256x which the softmax exp() absorbs
in its scale argument, Wo2 is host-divided by 16 to absorb the hidden
scale, and the bmat head-broadcast matrix carries 1/16 to unscale v.

The q*k product reads the q/k PSUM banks directly on DVE (no ACT
copies); v's PSUM->SBUF copies run on the otherwise idle GPSIMD. Head
reductions, the k-broadcast of qk, the sum over K, and the
head->channel broadcast of wv are matmuls against small constant
matrices. The Wo2 columns are host-permuted from (h,k,c) to (c,h,k)
order so the x/y coordinates occupy partition halves, making the
grid-sample weight product a single partition-offset vector multiply.
"""

import numpy as np
from contextlib import ExitStack

import ml_dtypes

import concourse.bacc as bacc
import concourse.tile as tile
from concourse import mybir

F32 = mybir.dt.float32
F32R = mybir.dt.float32r
F8 = mybir.dt.float8e4
AF = mybir.ActivationFunctionType
DR = mybir.MatmulPerfMode.DoubleRow
E4 = ml_dtypes.float8_e4m3

N, L, C, H, KP, D = 4, 16384, 256, 8, 4, 32
NCORES = 8
TOKS = N * L // NCORES  # 8192 tokens per core
TLOAD = 512             # tokens per DMA load tile
TCOMP = 512             # tokens per compute tile (PSUM free-dim limit, fp32)
ALPHA = 16.0            # host pre-scale on fp8-split weights
SIGMA = float(1.0 / np.sqrt(D))


def _build(toks=TOKS, tload=TLOAD, with_bias=False, psa=3, psb=5,
           hid_dve=True, order=0, v_late=False, prefetch=1,
           qkp2=False, m8=True, t2_act=False, ew_dve=True,
           last_split=0, dma_pool=False, early_load=True,
           out_pool=False, tail_split=1, spread0=True,
           store_act=False, spread1=False, fuse_s=False, kc_dve=False,
           ksb_bufs=1, hid_bufs=2, m_bufs=2, v_bufs=3, vlast=0,
           inp_bufs=4, psh=0, w_spread=True, split_store=True,
           hflast=1, drl=0, kfirst=False, ws=False, wb=3):
    nc = bacc.Bacc(trn_type="TRN2")
    dram = {}

    def din(name, shape, dt=None):
        dram[name] = nc.dram_tensor(name, list(shape), dt or F32R,
                                    kind="ExternalInput")
        return dram[name]

    xq = din("xq", (128, 2, 2, toks), F8)
    xk = din("xk", (128, 2, 2, toks), F8)
    xv = din("xv", (128, 2, 2, toks), F8)
    xp = din("xp", (128, 2, 2, toks), F8)
    ref = din("ref", (2, toks))
    din("wq", (128, 2, 2, 256), F8)
    din("wk", (128, 2, 2, 256), F8)
    din("wv", (128, 2, 2, 256), F8)
    din("wp", (128, 2, 2, 256), F8)
    din("wo1", (128, 2, 2, 512), F8)
    din("wo2", (128, 4, 64))
    din("wo", (128, 2, 256))
    din("bo1", (128, 4))
    din("bwof", (64, 1))
    din("smat", (64, 32))
    din("amat", (128, 2, 32), F8)
    din("cmat", (32, 8))
    din("cmat2", (64, 16))
    din("bmat", (8, 256))
    din("pmat", (2, 64))
    if with_bias:
        din("ones", (1, 512))
        din("bqp", (1, 256))
        din("bkp", (1, 256))
        din("bvr", (1, 256))
        din("bor", (1, 256))
    out = nc.dram_tensor("out", [128, 2, toks], F32, kind="ExternalOutput")

    nload = toks // tload
    nsub = tload // TCOMP
    T = TCOMP

    with tile.TileContext(nc) as tc, ExitStack() as ctx:
        singles = ctx.enter_context(tc.tile_pool(name="singles", bufs=1))
        inp = ctx.enter_context(tc.tile_pool(name="inp", bufs=inp_bufs))
        work = ctx.enter_context(tc.tile_pool(name="work", bufs=wb))
        psA = ctx.enter_context(tc.tile_pool(name="psA", bufs=psa, space="PSUM"))
        psB = ctx.enter_context(tc.tile_pool(name="psB", bufs=psb - psh,
                                             space="PSUM"))
        psH = (ctx.enter_context(tc.tile_pool(name="psH", bufs=psh,
                                              space="PSUM"))
               if psh else psB)

        _wq = [nc.sync, nc.scalar, nc.gpsimd]
        _wqi = [0]

        def load1(name, shape, dt=F32R):
            t = singles.tile(list(shape), dt, name=f"sb_{name}")
            eng = _wq[_wqi[0] % 3] if w_spread else nc.sync
            _wqi[0] += 1
            eng.dma_start(out=t, in_=dram[name][:])
            return t

        mm = nc.tensor.matmul

        def mm3(ps, w_s, x_t, s, mc, start, stop, lo_first=False, n=3):
            """fp8 DoubleRow accumulation: Wh*xh + Wh*xl (+ Wl*xh for n=3).
            lo_first emits the Wl term first so a preceding group ending on
            the same Wl stationary elides one LD_WEIGHTS."""
            m128 = slice(mc * 128, (mc + 1) * 128)
            terms = [(0, 0), (0, 1), (1, 0)][:n]
            if lo_first and n == 3:
                terms = [(1, 0), (0, 0), (0, 1)]
            for i, (wi, xi) in enumerate(terms):
                mm(ps, w_s[:, :, wi, m128], x_t[:, :, xi, s],
                   start=start and i == 0, stop=stop and i == len(terms) - 1,
                   perf_mode=DR)

        def load_tile(lt, spread=False):
            eng = nc.gpsimd if dma_pool else nc.sync
            e2 = nc.scalar if spread else eng
            e3 = nc.gpsimd if spread else eng
            t0 = lt * tload
            xv_t = inp.tile([128, 2, 2, tload], F8, tag="xv")
            eng.dma_start(out=xv_t, in_=xv[:, :, :, t0:t0 + tload])
            xq_t = inp.tile([128, 2, 2, tload], F8, tag="xq")
            e2.dma_start(out=xq_t, in_=xq[:, :, :, t0:t0 + tload])
            xp_t = inp.tile([128, 2, 2, tload], F8, tag="xp")
            e3.dma_start(out=xp_t, in_=xp[:, :, :, t0:t0 + tload])
            xk_t = inp.tile([128, 2, 2, tload], F8, tag="xk")
            eng.dma_start(out=xk_t, in_=xk[:, :, :, t0:t0 + tload])
            ref_t = inp.tile([2, tload], F32R, tag="ref")
            eng.dma_start(out=ref_t, in_=ref[:, t0:t0 + tload])
            return xq_t, xp_t, xk_t, xv_t, ref_t

        def emit_v(ld, lo, tz):
            xq_t, xp_t, xk_t, xv_t, ref_t = ld
            s = slice(lo, lo + tz)
            v_sb = work.tile([128, 2, tz], F32, tag="v", bufs=v_bufs)
            for mc in range(2):
                m128 = slice(mc * 128, (mc + 1) * 128)
                v_ps = psA.tile([128, tz], F32, tag="bigA")
                mm3(v_ps, wv_s, xv_t, s, mc, True, not with_bias)
                if with_bias:
                    mm(v_ps, bvr_s[:, m128], ones_s[:, :tz], start=False,
                       stop=True)
                nc.scalar.copy(v_sb[:, mc, :], v_ps)
            return v_sb

        def emit_hid_off(ld, lo, tz):
            xq_t, xp_t, xk_t, xv_t, ref_t = ld
            s = slice(lo, lo + tz)
            hid_sb = work.tile([128, 4, tz], F32R, tag="hid", bufs=hid_bufs)
            for j in range(4):
                h_ps = psH.tile([128, tz], F32,
                                tag="hps" if psh else "small")
                j128 = slice(j * 128, (j + 1) * 128)
                mm(h_ps, wo1_s[:, :, 0, j128], xq_t[:, :, 0, s], start=True,
                   stop=False, perf_mode=DR)
                mm(h_ps, wo1_s[:, :, 0, j128], xq_t[:, :, 1, s], start=False,
                   stop=False, perf_mode=DR)
                mm(h_ps, wo1_s[:, :, 1, j128], xq_t[:, :, 0, s], start=False,
                   stop=True, perf_mode=DR)
                if j == 3 and not with_bias and hid_dve:
                    nc.vector.tensor_scalar_max(hid_sb[:, j, :], h_ps, 0.0)
                else:
                    nc.scalar.activation(hid_sb[:, j, :], h_ps, AF.Relu,
                                         bias=bo1_s[:, j:j + 1], scale=1.0)
            off_ps = psB.tile([64, tz], F32, tag="small")
            for j in range(4):
                mm(off_ps, wo2_s[:, j, :], hid_sb[:, j, :],
                   start=(j == 0), stop=False)
            mm(off_ps, pmat_s, ref_t[:, s], start=False, stop=True)
            return off_ps

        def stage1(ld, lo, tz, vlate=None, hid_first=False):
            """Projection matmuls + q*k product + hidden/offset MLP."""
            vlate = v_late if vlate is None else vlate
            xq_t, xp_t, xk_t, xv_t, ref_t = ld
            s = slice(lo, lo + tz)
            if hid_first:
                off_ps = emit_hid_off(ld, lo, tz)
            if not vlate and not hid_first:
                v_sb = emit_v(ld, lo, tz)

            # q/k projections (+pos fused into the PSUM accumulation); k is
            # copied to SBUF on ACT (DVE may read only one PSUM operand) and
            # the q*k product reads the q PSUM bank directly on DVE
            m_sb = work.tile([128, 2, tz], F8 if m8 else F32R, tag="m",
                             bufs=m_bufs)
            k_sb = work.tile([128, 2, tz], F32, tag="ksb", bufs=ksb_bufs)
            for mc in range(2):
                m128 = slice(mc * 128, (mc + 1) * 128)
                nq = 2 if qkp2 else 3
                if kfirst:
                    k_ps = psA.tile([128, tz], F32, tag="bigA")
                    mm3(k_ps, wk_s, xk_t, s, mc, True, False, n=nq)
                    mm3(k_ps, wp_s, xp_t, s, mc, False, not with_bias, n=nq)
                    if with_bias:
                        mm(k_ps, bkp_s[:, m128], ones_s[:, :tz], start=False,
                           stop=True)
                    q_ps = psA.tile([128, tz], F32, tag="bigA")
                    mm3(q_ps, wp_s, xp_t, s, mc, True, False, lo_first=True,
                        n=nq)
                    mm3(q_ps, wq_s, xq_t, s, mc, False, not with_bias, n=nq)
                    if with_bias:
                        mm(q_ps, bqp_s[:, m128], ones_s[:, :tz], start=False,
                           stop=True)
                else:
                    q_ps = psA.tile([128, tz], F32, tag="bigA")
                    mm3(q_ps, wq_s, xq_t, s, mc, True, False, n=nq)
                    mm3(q_ps, wp_s, xp_t, s, mc, False, not with_bias, n=nq)
                    if with_bias:
                        mm(q_ps, bqp_s[:, m128], ones_s[:, :tz], start=False,
                           stop=True)
                    k_ps = psA.tile([128, tz], F32, tag="bigA")
                    # Wpos first (lo term leading for 3-term) so q's trailing
                    # Wpos stationary carries over without a reload
                    mm3(k_ps, wp_s, xp_t, s, mc, True, False, lo_first=True,
                        n=nq)
                    mm3(k_ps, wk_s, xk_t, s, mc, False, not with_bias, n=nq)
                    if with_bias:
                        mm(k_ps, bkp_s[:, m128], ones_s[:, :tz], start=False,
                           stop=True)
                if kc_dve:
                    nc.vector.tensor_copy(k_sb[:, mc, :], k_ps)
                else:
                    nc.scalar.copy(k_sb[:, mc, :], k_ps)
                if m8:
                    # m = (q/256)*k in fp8 -- unscales the 16x on q and k so
                    # fp8's +-448 range holds the products
                    nc.vector.scalar_tensor_tensor(
                        m_sb[:, mc, :], q_ps, 1.0 / 256.0, k_sb[:, mc, :],
                        op0=mybir.AluOpType.mult, op1=mybir.AluOpType.mult)
                else:
                    nc.vector.tensor_mul(m_sb[:, mc, :], q_ps, k_sb[:, mc, :])
            if vlate:
                v_sb = emit_v(ld, lo, tz)
            if hid_first:
                v_sb = emit_v(ld, lo, tz)
            else:
                if vlate:
                    v_sb = emit_v(ld, lo, tz)
                off_ps = emit_hid_off(ld, lo, tz)
            return m_sb, v_sb, off_ps, tz

        def stage2a_w(state):
            """Grid-sample weight w from the offsets (depends only on
            stage1's off, so it can run in the same iteration)."""
            m_sb, v_sb, off_ps, tz = state
            t1_sb = work.tile([64, tz], F32, tag="t1")
            nc.scalar.activation(t1_sb, off_ps[:, :], AF.Abs, bias=bwof_s,
                                 scale=1.0)
            t2_sb = work.tile([64, tz], F32R, tag="t2")
            if t2_act:
                nc.scalar.activation(t2_sb, t1_sb, AF.Relu, bias=1.0,
                                     scale=-1.0)
            else:
                nc.gpsimd.tensor_scalar(t2_sb, t1_sb, 1.0, 1.0,
                                        mybir.AluOpType.min,
                                        mybir.AluOpType.subtract)
            t2y_ps = psB.tile([32, tz], F32, tag="small")
            mm(t2y_ps, smat_s, t2_sb, start=True, stop=True)
            w_sb = work.tile([32, tz], F32, tag="w")
            nc.vector.tensor_mul(w_sb, t2_sb[0:32, :], t2y_ps)
            return m_sb, v_sb, w_sb, tz

        def stage2a_rest(wstate, o0=0, osz=None, drain=False):
            """qk head-sum, softmax partial sums (w precomputed)."""
            m_sb, v_sb, w_all, tz = wstate
            if osz is None:
                osz = tz
            so = slice(o0, o0 + osz)
            tz = osz
            w_sb = w_all[:, so]

            qk_ps = psB.tile([32, tz], F32, tag="small")
            if m8:
                mm(qk_ps, amat_s, m_sb[:, :, so], start=True, stop=True,
                   perf_mode=DR)
            else:
                mm(qk_ps, amat_s[:, 0:32], m_sb[:, 0, so], start=True,
                   stop=False)
                mm(qk_ps, amat_s[:, 32:64], m_sb[:, 1, so], start=False,
                   stop=True)
            lg_sb = work.tile([32, tz], F32, tag="lg")
            nc.vector.tensor_mul(lg_sb, qk_ps, w_sb)
            escale = SIGMA if m8 else SIGMA / (ALPHA * ALPHA)
            e_sb = work.tile([32, tz], F32R, tag="e")
            nc.scalar.activation(e_sb, lg_sb, AF.Exp, bias=0.0, scale=escale)
            s1_ps = psB.tile([8, tz], F32, tag="small")
            mm(s1_ps, cmat_s, e_sb, start=True, stop=True)
            ew_sb = work.tile([32, tz], F32R, tag="ew")
            if ew_dve and not drain:
                nc.vector.tensor_mul(ew_sb, e_sb, w_sb)
            else:
                nc.gpsimd.tensor_mul(ew_sb, e_sb, w_sb)
            s2_ps = psB.tile([8, tz], F32, tag="small")
            mm(s2_ps, cmat_s, ew_sb, start=True, stop=True)
            return s1_ps, s2_ps, v_sb, tz, o0

        def stage2a(state, o0=0, osz=None, drain=False):
            """Head-sum of q*k, grid-sample weight w, softmax partial sums.
            o0/osz optionally restrict to a token sub-range of the tile
            (used to pipeline the final tile's drain)."""
            m_sb, v_sb, off_ps, tz = state
            if osz is None:
                osz = tz
            so = slice(o0, o0 + osz)
            tz = osz

            qk_ps = psB.tile([32, tz], F32, tag="small")
            if m8:
                mm(qk_ps, amat_s, m_sb[:, :, so], start=True, stop=True,
                   perf_mode=DR)
            else:
                mm(qk_ps, amat_s[:, 0:32], m_sb[:, 0, so], start=True,
                   stop=False)
                mm(qk_ps, amat_s[:, 32:64], m_sb[:, 1, so], start=False,
                   stop=True)

            # w = relu(1-|sp_x-.5|)*relu(1-|sp_y-.5|); computed sign-flipped
            # as t2n = min(t1,1)-1 = -relu(1-t1) on GPSIMD (one tensor_scalar)
            # -- the two negative factors cancel in the product. The y half
            # is moved to partitions 0-31 with a PE row-select matmul since
            # DVE can't pair operands at different base partitions.
            t1_sb = work.tile([64, tz], F32, tag="t1")
            nc.scalar.activation(t1_sb, off_ps[:, so], AF.Abs, bias=bwof_s,
                                 scale=1.0)
            t2_sb = work.tile([64, tz], F32R, tag="t2")
            if t2_act:
                nc.scalar.activation(t2_sb, t1_sb, AF.Relu, bias=1.0,
                                     scale=-1.0)
            else:
                nc.gpsimd.tensor_scalar(t2_sb, t1_sb, 1.0, 1.0,
                                        mybir.AluOpType.min,
                                        mybir.AluOpType.subtract)
            t2y_ps = psB.tile([32, tz], F32, tag="small")
            mm(t2y_ps, smat_s, t2_sb, start=True, stop=True)
            w_sb = work.tile([32, tz], F32, tag="w")
            nc.vector.tensor_mul(w_sb, t2_sb[0:32, :], t2y_ps)

            # softmax over K: e = exp(qk*w/(256*sqrt(D))) -- the 1/256
            # unscales the 16x on each of q and k; the qk head-sum is read
            # straight from PSUM; e*w runs on the otherwise idle GPSIMD
            lg_sb = work.tile([32, tz], F32, tag="lg")
            nc.vector.tensor_mul(lg_sb, qk_ps, w_sb)
            escale = SIGMA if m8 else SIGMA / (ALPHA * ALPHA)
            if fuse_s:
                # e and e*w stacked in one [64,T] tile -> ONE [64->16]
                # matmul yields both softmax partial sums
                eew_sb = work.tile([64, tz], F32R, tag="eew")
                nc.scalar.activation(eew_sb[0:32, :], lg_sb, AF.Exp,
                                     bias=0.0, scale=escale)
                nc.vector.tensor_mul(eew_sb[32:64, :], eew_sb[0:32, :], w_sb)
                s12_ps = psB.tile([16, tz], F32, tag="small")
                mm(s12_ps, cmat2_s, eew_sb, start=True, stop=True)
                s1_ps, s2_ps = s12_ps[0:8, :], s12_ps[8:16, :]
            else:
                e_sb = work.tile([32, tz], F32R, tag="e")
                nc.scalar.activation(e_sb, lg_sb, AF.Exp, bias=0.0,
                                     scale=escale)
                # s1 fires immediately after exp so recip overlaps the e*w
                # product feeding s2
                s1_ps = psB.tile([8, tz], F32, tag="small")
                mm(s1_ps, cmat_s, e_sb, start=True, stop=True)
                ew_sb = work.tile([32, tz], F32R, tag="ew")
                if ew_dve and not drain:
                    nc.vector.tensor_mul(ew_sb, e_sb, w_sb)
                else:
                    nc.gpsimd.tensor_mul(ew_sb, e_sb, w_sb)
                s2_ps = psB.tile([8, tz], F32, tag="small")
                mm(s2_ps, cmat_s, ew_sb, start=True, stop=True)
            return s1_ps, s2_ps, v_sb, tz, o0

        def stage2b(state, g0, drain=False):
            """Softmax normalization, ov = v*wv, out-projection, store."""
            s1_ps, s2_ps, v_sb, tz, o0 = state
            so = slice(o0, o0 + tz)
            r1_sb = work.tile([8, tz], F32, tag="r1")
            nc.vector.reciprocal(r1_sb, s1_ps)
            wv_sb = work.tile([8, tz], F32R, tag="wvv")
            nc.vector.tensor_mul(wv_sb, s2_ps, r1_sb)

            # ov = v * wv (broadcast head->channels via matmul; bmat carries
            # 1/16 to unscale the 16x on v)
            ov_sb = work.tile([128, 2, tz], F32R, tag="ov")
            for mc in range(2):
                wvx_ps = psB.tile([128, tz], F32, tag="small")
                mm(wvx_ps, bmat_s[:, mc * 128:(mc + 1) * 128], wv_sb,
                   start=True, stop=True)
                nc.vector.tensor_mul(ov_sb[:, mc, :], v_sb[:, mc, so],
                                     wvx_ps)

            # outT = Wout.T @ ov, channel-major [256, T] (stationary = Wout
            # chunks: 4 LD_WEIGHTS instead of 8); host transposes back
            o_sb = work.tile([128, 2, tz], F32, tag="osb")
            for oc in range(2):
                o_ps = psB.tile([128, tz], F32, tag="small")
                oc128 = slice(oc * 128, (oc + 1) * 128)
                mm(o_ps, wo_s[:, 0, oc128], ov_sb[:, 0, :], start=True,
                   stop=False)
                mm(o_ps, wo_s[:, 1, oc128], ov_sb[:, 1, :], start=False,
                   stop=not with_bias)
                if with_bias:
                    mm(o_ps, bor_s[0:1, oc128], ones_s[0:1, :tz],
                       start=False, stop=True)
                if oc == 0 and not drain:
                    nc.vector.tensor_copy(o_sb[:, oc, :], o_ps)
                else:
                    nc.scalar.copy(o_sb[:, oc, :], o_ps)
            oeng = (nc.gpsimd if out_pool
                    else nc.scalar if store_act else nc.sync)
            if split_store:
                for oc in range(2):
                    oeng.dma_start(out=out[:, oc, g0 + o0:g0 + o0 + tz],
                                   in_=o_sb[:, oc, :])
            else:
                oeng.dma_start(out=out[:, :, g0 + o0:g0 + o0 + tz], in_=o_sb)

        # 3-deep software pipeline: per iteration emit tile i's matmul-heavy
        # stage1, then tile i-2's output tail (stage2b), then tile i-1's
        # softmax chain (stage2a) — PE stays dense while ACT/DVE chains of
        # earlier tiles drain. stage2b(i-2) must precede stage2a(i-1) so the
        # s1/s2 PSUM slots recycle in trace order.
        assert nsub in (1, 2)
        units = []
        for lt in range(nload):
            for lo in range(0, tload, TCOMP):
                if lt >= nload - last_split:
                    units.append((lt, lo, TCOMP // 2))
                    units.append((lt, lo + TCOMP // 2, TCOMP // 2))
                else:
                    units.append((lt, lo, TCOMP))
        p1 = p2 = None  # (state, g0) for stage2a / stage2b
        # first input tile before the weights so the PE can start ASAP;
        # weights ordered by first use
        wv_s = load1("wv", (128, 2, 2, 256), F8)
        ld = load_tile(0, spread=spread0)
        wq_s = load1("wq", (128, 2, 2, 256), F8)
        wp_s = load1("wp", (128, 2, 2, 256), F8)
        wk_s = load1("wk", (128, 2, 2, 256), F8)
        if early_load and nload > 1:
            if spread1:
                _sv = nc.sync
                nc.sync = nc.gpsimd
                ld1 = load_tile(1)
                nc.sync = _sv
            else:
                ld1 = load_tile(1)
        else:
            ld1 = None
        wo1_s = load1("wo1", (128, 2, 2, 512), F8)
        bo1_s = load1("bo1", (128, 4))
        amat_s = load1("amat", (128, 2, 32), F8)
        wo2_s = load1("wo2", (128, 4, 64))
        pmat_s = load1("pmat", (2, 64))
        bwof_s = load1("bwof", (64, 1))
        smat_s = load1("smat", (64, 32))
        cmat_s = load1("cmat", (32, 8))
        cmat2_s = load1("cmat2", (64, 16)) if fuse_s else None
        bmat_s = load1("bmat", (8, 256))
        wo_s = load1("wo", (128, 2, 256))
        if with_bias:
            bqp_s = load1("bqp", (1, 256))
            bkp_s = load1("bkp", (1, 256))
            bvr_s = load1("bvr", (1, 256))
            bor_s = load1("bor", (1, 256))
            ones_s = load1("ones", (1, 512))
        lds = {0: ld}
        if early_load and nload > 1:
            lds[1] = ld1
        for pf in range(1, min(prefetch, nload)):
            if pf not in lds:
                lds[pf] = load_tile(pf)
        for ui, (lt, lo, tz) in enumerate(units):
            nxt = ui + prefetch
            if nxt < len(units) and units[nxt][0] not in lds:
                lds[units[nxt][0]] = load_tile(units[nxt][0])
            ld = lds[lt]
            vl = True if (len(units) - ui) <= vlast else None
            hf = (len(units) - ui) <= hflast
            if order == 0:
                state = stage1(ld, lo, tz, vlate=vl, hid_first=hf)
                if ws:
                    state = stage2a_w(state)
                if p2 is not None:
                    stage2b(*p2)
                    p2 = None
                if p1 is not None:
                    st2, g0p = p1
                    p2 = ((stage2a_rest(st2) if ws else stage2a(st2)), g0p)
            elif order == 1:
                if p2 is not None:
                    stage2b(*p2)
                    p2 = None
                state = stage1(ld, lo, tz)
                if p1 is not None:
                    st2, g0p = p1
                    p2 = (stage2a(st2), g0p)
            elif order == 2:
                if p2 is not None:
                    stage2b(*p2)
                    p2 = None
                if p1 is not None:
                    st2, g0p = p1
                    p2 = (stage2a(st2), g0p)
                state = stage1(ld, lo, tz)
            else:  # order == 3: 2-deep, 2a(i) right in iteration i
                state = stage1(ld, lo, tz)
                if p2 is not None:
                    stage2b(*p2)
                    p2 = None
                p2 = (stage2a((state, )[0]), lt * tload + lo)
            p1 = (state, lt * tload + lo)
        if p2 is not None:
            stage2b(*p2)
        if order != 3:
            st2, g0p = p1
            ftz = st2[3]
            nchunk = max(1, tail_split)
            csz = ftz // nchunk
            dr = drl > 0
            s2a = stage2a_rest if ws else stage2a
            parts = [s2a(st2, o0=ci * csz, osz=csz, drain=dr)
                     for ci in range(nchunk)]
            for part in parts:
                stage2b(part, g0p, drain=dr)

    nc.compile()
    return nc


def _consts():
    amat = np.zeros((128, 64), np.float32)
    for mc in range(2):
        for d in range(128):
            h = mc * 4 + d // 32
            for k in range(KP):
                amat[d, mc * 32 + h * KP + k] = 1.0
    cmat = np.zeros((32, 8), np.float32)
    for j in range(32):
        cmat[j, j // KP] = 1.0
    cmat2 = np.zeros((64, 16), np.float32)
    for j in range(32):
        cmat2[j, j // KP] = 1.0
        cmat2[32 + j, 8 + j // KP] = 1.0
    bmat = np.zeros((8, 256), np.float32)
    for mc in range(2):
        for c in range(128):
            bmat[mc * 4 + c // 32, mc * 128 + c] = 1.0 / ALPHA
    pmat = np.zeros((2, 64), np.float32)
    for r in range(64):
        pmat[r // 32, r] = 1.0
    smat = np.zeros((64, 32), np.float32)
    for j in range(32):
        smat[32 + j, j] = 1.0
    return amat, cmat, cmat2, bmat, pmat, smat


def _wsplit(w):
    # [256, O] -> [128, 2, O]  (row kc*128+p  ->  [p, kc, :])
    o = w.shape[1]
    return np.ascontiguousarray(w.reshape(2, 128, o).transpose(1, 0, 2))


def _xsplit(x):
    # [T, 256] token-major -> [128, 2, T] channel-major chunks
    t = x.shape[0]
    return np.ascontiguousarray(x.T.reshape(2, 128, t).transpose(1, 0, 2))


def _split8(a):
    # [128, 2, O] f32 -> [128, 2, 2, O] fp8 (dim2 = hi, lo of the value)
    hi = a.astype(E4)
    lo = (a - hi.astype(np.float32)).astype(E4)
    return np.ascontiguousarray(np.stack([hi, lo], axis=2))


def _host_maps(inputs, toks, ncores):
    f32 = lambda v: np.asarray(v, dtype=np.float32)
    query = f32(inputs["query"]).reshape(-1, C)
    key = f32(inputs["key"]).reshape(-1, C)
    value = f32(inputs["value"]).reshape(-1, C)
    pos = f32(inputs["pos_embed"]).reshape(-1, C)
    refp = f32(inputs["reference_points"]).reshape(-1, 2)

    # permute Wo2 columns (h,k,c) -> (c,h,k)
    perm = [h * (KP * 2) + k * 2 + c for c in range(2) for h in range(H)
            for k in range(KP)]
    wo2p = f32(inputs["Wo2"])[:, perm]
    bo2p = f32(inputs["bo2"])[perm]

    amat, cmat, cmat2, bmat, pmat, smat = _consts()
    bqp = f32(inputs["bq"]) + f32(inputs["bpos"])
    bkp = f32(inputs["bk"]) + f32(inputs["bpos"])
    bv = f32(inputs["bv"])
    bout = f32(inputs["bout"])
    with_bias = any(np.any(b != 0) for b in (bqp, bkp, bv, bout))

    # Wo2 operand (hidden) carries 16x -> divide Wo2 by 16 to compensate
    wo2r = np.ascontiguousarray(
        (wo2p / ALPHA).reshape(4, 128, 64).transpose(1, 0, 2))
    shared = {
        "wq": _split8(_wsplit(ALPHA * f32(inputs["Wq"]))),
        "wk": _split8(_wsplit(ALPHA * f32(inputs["Wk"]))),
        "wv": _split8(_wsplit(ALPHA * f32(inputs["Wv"]))),
        "wp": _split8(_wsplit(ALPHA * f32(inputs["Wpos"]))),
        "wo1": _split8(_wsplit(ALPHA * f32(inputs["Wo1"]))),
        "wo2": wo2r,
        "wo": _wsplit(f32(inputs["Wout"])),
        "bo1": np.ascontiguousarray(
            (ALPHA * f32(inputs["bo1"])).reshape(4, 128).T),
        "bwof": np.ascontiguousarray((bo2p - 0.5).reshape(64, 1)),
        "smat": smat,
        "amat": np.ascontiguousarray(amat.reshape(128, 2, 32)).astype(E4),
        "cmat": cmat, "cmat2": cmat2, "bmat": bmat, "pmat": pmat,
    }
    if with_bias:
        shared["ones"] = np.ones((1, 512), np.float32)
        shared["bqp"] = ALPHA * bqp.reshape(1, 256)
        shared["bkp"] = ALPHA * bkp.reshape(1, 256)
        shared["bvr"] = ALPHA * bv.reshape(1, 256)
        shared["bor"] = bout.reshape(1, 256)

    in_maps = []
    for cid in range(ncores):
        sl = slice(cid * toks, (cid + 1) * toks)
        m = dict(shared)
        m["xq"] = _split8(_xsplit(query[sl]))
        m["xk"] = _split8(_xsplit(key[sl]))
        m["xv"] = _split8(_xsplit(value[sl]))
        m["xp"] = _split8(_xsplit(pos[sl]))
        m["ref"] = np.ascontiguousarray(refp[sl].T)
        in_maps.append(m)
    return in_maps, with_bias


_NC_CACHE = {}


def kernel(**inputs):
    from concourse.bass_utils import run_bass_kernel_spmd

    in_maps, with_bias = _host_maps(inputs, TOKS, NCORES)
    ck = ("full", with_bias)
    if ck not in _NC_CACHE:
        _NC_CACHE[ck] = _build(toks=TOKS, tload=TLOAD, with_bias=with_bias)
    nc = _NC_CACHE[ck]
    res = run_bass_kernel_spmd(nc, in_maps, core_ids=list(range(NCORES)))
    # out is channel-major [128, 2, toks]: out[t, kc*128+p] = o[p, kc, t]
    outs = [np.asarray(r["out"]).transpose(2, 1, 0).reshape(TOKS, C)
            for r in res.results]
    full = np.concatenate(outs, axis=0).reshape(N, L, C)
    return np.ascontiguousarray(full.astype(np.float32))


# revision 66
# speedup vs baseline: 1.0007x; 1.0002x over previous
"""Trainium2 Bass kernel for ExtensibleAttention (sparse_attention).

Strategy: data-parallel over the 65536 tokens (N*L flattened) across 8
NeuronCores; the small 256-dim projection weights are replicated. All
per-token math is fused into one pass per 512-token tile.

The q/k/pos/v/Wo1 projections run as fp8e4m3 DoubleRow matmuls (2
contraction slices per instruction at 0.5 cycles/row = 4x fp32r rate).
Accuracy is recovered with a 3-term split: activations are shipped as an
fp8 (hi, lo) pair, weights are pre-scaled by 16 on the host (so their
fp8 residual stays out of e4m3's subnormal range) and also split, and
the kernel accumulates Wh*xh + Wh*xl + Wl*xh in PSUM -> ~8e-4 rms error
per matmul, better than bf16. The x16 weight scale is folded downstream
for free: the q*k product carries 256x which the softmax exp() absorbs
in its scale argument, Wo2 is host-divided by 16 to absorb the hidden
scale, and the bmat head-broadcast matrix carries 1/16 to unscale v.

The q*k product reads the q/k PSUM banks directly on DVE (no ACT
copies); v's PSUM->SBUF copies run on the otherwise idle GPSIMD. Head
reductions, the k-broadcast of qk, the sum over K, and the
head->channel broadcast of wv are matmuls against small constant
matrices. The Wo2 columns are host-permuted from (h,k,c) to (c,h,k)
order so the x/y coordinates occupy partition halves, making the
grid-sample weight product a single partition-offset vector multiply.
"""

import numpy as np
from contextlib import ExitStack

import ml_dtypes

import concourse.bacc as bacc
import concourse.tile as tile
from concourse import mybir

F32 = mybir.dt.float32
F32R = mybir.dt.float32r
F8 = mybir.dt.float8e4
AF = mybir.ActivationFunctionType
DR = mybir.MatmulPerfMode.DoubleRow
E4 = ml_dtypes.float8_e4m3

N, L, C, H, KP, D = 4, 16384, 256, 8, 4, 32
NCORES = 8
TOKS = N * L // NCORES  # 8192 tokens per core
TLOAD = 512             # tokens per DMA load tile
TCOMP = 512             # tokens per compute tile (PSUM free-dim limit, fp32)
ALPHA = 16.0            # host pre-scale on fp8-split weights
SIGMA = float(1.0 / np.sqrt(D))


def _build(toks=TOKS, tload=TLOAD, with_bias=False, psa=3, psb=5,
           hid_dve=True, order=0, v_late=False, prefetch=1,
           qkp2=False, m8=True, t2_act=False, ew_dve=True,
           last_split=0, dma_pool=False, early_load=True,
           out_pool=False, tail_split=1, spread0=True,
           store_act=False, spread1=False, fuse_s=False, kc_dve=False,
           ksb_bufs=1, hid_bufs=2, m_bufs=2, v_bufs=3, vlast=0,
           inp_bufs=4, psh=0, w_spread=True, split_store=True,
           hflast=1, drl=0, kfirst=False, ws=False, wb=3,
           ref_pre=4):
    nc = bacc.Bacc(trn_type="TRN2")
    dram = {}

    def din(name, shape, dt=None):
        dram[name] = nc.dram_tensor(name, list(shape), dt or F32R,
                                    kind="ExternalInput")
        return dram[name]

    xq = din("xq", (128, 2, 2, toks), F8)
    xk = din("xk", (128, 2, 2, toks), F8)
    xv = din("xv", (128, 2, 2, toks), F8)
    xp = din("xp", (128, 2, 2, toks), F8)
    ref = din("ref", (2, toks))
    din("wq", (128, 2, 2, 256), F8)
    din("wk", (128, 2, 2, 256), F8)
    din("wv", (128, 2, 2, 256), F8)
    din("wp", (128, 2, 2, 256), F8)
    din("wo1", (128, 2, 2, 512), F8)
    din("wo2", (128, 4, 64))
    din("wo", (128, 2, 256))
    din("bo1", (128, 4))
    din("bwof", (64, 1))
    din("smat", (64, 32))
    din("amat", (128, 2, 32), F8)
    din("cmat", (32, 8))
    din("cmat2", (64, 16))
    din("bmat", (8, 256))
    din("pmat", (2, 64))
    if with_bias:
        din("ones", (1, 512))
        din("bqp", (1, 256))
        din("bkp", (1, 256))
        din("bvr", (1, 256))
        din("bor", (1, 256))
    out = nc.dram_tensor("out", [128, 2, toks], F32, kind="ExternalOutput")

    nload = toks // tload
    nsub = tload // TCOMP
    T = TCOMP

    with tile.TileContext(nc) as tc, ExitStack() as ctx:
        singles = ctx.enter_context(tc.tile_pool(name="singles", bufs=1))
        inp = ctx.enter_context(tc.tile_pool(name="inp", bufs=inp_bufs))
        work = ctx.enter_context(tc.tile_pool(name="work", bufs=wb))
        psA = ctx.enter_context(tc.tile_pool(name="psA", bufs=psa, space="PSUM"))
        psB = ctx.enter_context(tc.tile_pool(name="psB", bufs=psb - psh,
                                             space="PSUM"))
        psH = (ctx.enter_context(tc.tile_pool(name="psH", bufs=psh,
                                              space="PSUM"))
               if psh else psB)

        _wq = [nc.sync, nc.scalar, nc.gpsimd]
        _wqi = [0]

        def load1(name, shape, dt=F32R):
            t = singles.tile(list(shape), dt, name=f"sb_{name}")
            eng = _wq[_wqi[0] % 3] if w_spread else nc.sync
            _wqi[0] += 1
            eng.dma_start(out=t, in_=dram[name][:])
            return t

        mm = nc.tensor.matmul

        def mm3(ps, w_s, x_t, s, mc, start, stop, lo_first=False, n=3):
            """fp8 DoubleRow accumulation: Wh*xh + Wh*xl (+ Wl*xh for n=3).
            lo_first emits the Wl term first so a preceding group ending on
            the same Wl stationary elides one LD_WEIGHTS."""
            m128 = slice(mc * 128, (mc + 1) * 128)
            terms = [(0, 0), (0, 1), (1, 0)][:n]
            if lo_first and n == 3:
                terms = [(1, 0), (0, 0), (0, 1)]
            for i, (wi, xi) in enumerate(terms):
                mm(ps, w_s[:, :, wi, m128], x_t[:, :, xi, s],
                   start=start and i == 0, stop=stop and i == len(terms) - 1,
                   perf_mode=DR)

        _refs = {}

        def load_tile(lt, spread=False):
            eng = nc.gpsimd if dma_pool else nc.sync
            e2 = nc.scalar if spread else eng
            e3 = nc.gpsimd if spread else eng
            t0 = lt * tload
            xv_t = inp.tile([128, 2, 2, tload], F8, tag="xv")
            eng.dma_start(out=xv_t, in_=xv[:, :, :, t0:t0 + tload])
            xq_t = inp.tile([128, 2, 2, tload], F8, tag="xq")
            e2.dma_start(out=xq_t, in_=xq[:, :, :, t0:t0 + tload])
            xp_t = inp.tile([128, 2, 2, tload], F8, tag="xp")
            e3.dma_start(out=xp_t, in_=xp[:, :, :, t0:t0 + tload])
            xk_t = inp.tile([128, 2, 2, tload], F8, tag="xk")
            eng.dma_start(out=xk_t, in_=xk[:, :, :, t0:t0 + tload])
            rg = max(1, int(ref_pre))  # tiles per batched ref DMA
            if lt % rg == 0:
                rt = inp.tile([2, rg * tload], F32R, tag="ref")
                eng.dma_start(
                    out=rt, in_=ref[:, t0:min(t0 + rg * tload, toks)])
                _refs[lt // rg] = rt
            base = (lt // rg) * rg
            ref_t = _refs[lt // rg][:, (lt - base) * tload:
                                    (lt - base + 1) * tload]
            return xq_t, xp_t, xk_t, xv_t, ref_t

        def emit_v(ld, lo, tz):
            xq_t, xp_t, xk_t, xv_t, ref_t = ld
            s = slice(lo, lo + tz)
            v_sb = work.tile([128, 2, tz], F32, tag="v", bufs=v_bufs)
            for mc in range(2):
                m128 = slice(mc * 128, (mc + 1) * 128)
                v_ps = psA.tile([128, tz], F32, tag="bigA")
                mm3(v_ps, wv_s, xv_t, s, mc, True, not with_bias)
                if with_bias:
                    mm(v_ps, bvr_s[:, m128], ones_s[:, :tz], start=False,
                       stop=True)
                nc.scalar.copy(v_sb[:, mc, :], v_ps)
            return v_sb

        def emit_hid_off(ld, lo, tz):
            xq_t, xp_t, xk_t, xv_t, ref_t = ld
            s = slice(lo, lo + tz)
            hid_sb = work.tile([128, 4, tz], F32R, tag="hid", bufs=hid_bufs)
            for j in range(4):
                h_ps = psH.tile([128, tz], F32,
                                tag="hps" if psh else "small")
                j128 = slice(j * 128, (j + 1) * 128)
                mm(h_ps, wo1_s[:, :, 0, j128], xq_t[:, :, 0, s], start=True,
                   stop=False, perf_mode=DR)
                mm(h_ps, wo1_s[:, :, 0, j128], xq_t[:, :, 1, s], start=False,
                   stop=False, perf_mode=DR)
                mm(h_ps, wo1_s[:, :, 1, j128], xq_t[:, :, 0, s], start=False,
                   stop=True, perf_mode=DR)
                if j == 3 and not with_bias and hid_dve:
                    nc.vector.tensor_scalar_max(hid_sb[:, j, :], h_ps, 0.0)
                else:
                    nc.scalar.activation(hid_sb[:, j, :], h_ps, AF.Relu,
                                         bias=bo1_s[:, j:j + 1], scale=1.0)
            off_ps = psB.tile([64, tz], F32, tag="small")
            for j in range(4):
                mm(off_ps, wo2_s[:, j, :], hid_sb[:, j, :],
                   start=(j == 0), stop=False)
            mm(off_ps, pmat_s, ref_t[:, s], start=False, stop=True)
            return off_ps

        def stage1(ld, lo, tz, vlate=None, hid_first=False):
            """Projection matmuls + q*k product + hidden/offset MLP."""
            vlate = v_late if vlate is None else vlate
            xq_t, xp_t, xk_t, xv_t, ref_t = ld
            s = slice(lo, lo + tz)
            if hid_first:
                off_ps = emit_hid_off(ld, lo, tz)
            if not vlate and not hid_first:
                v_sb = emit_v(ld, lo, tz)

            # q/k projections (+pos fused into the PSUM accumulation); k is
            # copied to SBUF on ACT (DVE may read only one PSUM operand) and
            # the q*k product reads the q PSUM bank directly on DVE
            m_sb = work.tile([128, 2, tz], F8 if m8 else F32R, tag="m",
                             bufs=m_bufs)
            k_sb = work.tile([128, 2, tz], F32, tag="ksb", bufs=ksb_bufs)
            for mc in range(2):
                m128 = slice(mc * 128, (mc + 1) * 128)
                nq = 2 if qkp2 else 3
                if kfirst:
                    k_ps = psA.tile([128, tz], F32, tag="bigA")
                    mm3(k_ps, wk_s, xk_t, s, mc, True, False, n=nq)
                    mm3(k_ps, wp_s, xp_t, s, mc, False, not with_bias, n=nq)
                    if with_bias:
                        mm(k_ps, bkp_s[:, m128], ones_s[:, :tz], start=False,
                           stop=True)
                    q_ps = psA.tile([128, tz], F32, tag="bigA")
                    mm3(q_ps, wp_s, xp_t, s, mc, True, False, lo_first=True,
                        n=nq)
                    mm3(q_ps, wq_s, xq_t, s, mc, False, not with_bias, n=nq)
                    if with_bias:
                        mm(q_ps, bqp_s[:, m128], ones_s[:, :tz], start=False,
                           stop=True)
                else:
                    q_ps = psA.tile([128, tz], F32, tag="bigA")
                    mm3(q_ps, wq_s, xq_t, s, mc, True, False, n=nq)
                    mm3(q_ps, wp_s, xp_t, s, mc, False, not with_bias, n=nq)
                    if with_bias:
                        mm(q_ps, bqp_s[:, m128], ones_s[:, :tz], start=False,
                           stop=True)
                    k_ps = psA.tile([128, tz], F32, tag="bigA")
                    # Wpos first (lo term leading for 3-term) so q's trailing
                    # Wpos stationary carries over without a reload
                    mm3(k_ps, wp_s, xp_t, s, mc, True, False, lo_first=True,
                        n=nq)
                    mm3(k_ps, wk_s, xk_t, s, mc, False, not with_bias, n=nq)
                    if with_bias:
                        mm(k_ps, bkp_s[:, m128], ones_s[:, :tz], start=False,
                           stop=True)
                if kc_dve:
                    nc.vector.tensor_copy(k_sb[:, mc, :], k_ps)
                else:
                    nc.scalar.copy(k_sb[:, mc, :], k_ps)
                if m8:
                    # m = (q/256)*k in fp8 -- unscales the 16x on q and k so
                    # fp8's +-448 range holds the products
                    nc.vector.scalar_tensor_tensor(
                        m_sb[:, mc, :], q_ps, 1.0 / 256.0, k_sb[:, mc, :],
                        op0=mybir.AluOpType.mult, op1=mybir.AluOpType.mult)
                else:
                    nc.vector.tensor_mul(m_sb[:, mc, :], q_ps, k_sb[:, mc, :])
            if vlate:
                v_sb = emit_v(ld, lo, tz)
            if hid_first:
                v_sb = emit_v(ld, lo, tz)
            else:
                if vlate:
                    v_sb = emit_v(ld, lo, tz)
                off_ps = emit_hid_off(ld, lo, tz)
            return m_sb, v_sb, off_ps, tz

        def stage2a_w(state):
            """Grid-sample weight w from the offsets (depends only on
            stage1's off, so it can run in the same iteration)."""
            m_sb, v_sb, off_ps, tz = state
            t1_sb = work.tile([64, tz], F32, tag="t1")
            nc.scalar.activation(t1_sb, off_ps[:, :], AF.Abs, bias=bwof_s,
                                 scale=1.0)
            t2_sb = work.tile([64, tz], F32R, tag="t2")
            if t2_act:
                nc.scalar.activation(t2_sb, t1_sb, AF.Relu, bias=1.0,
                                     scale=-1.0)
            else:
                nc.gpsimd.tensor_scalar(t2_sb, t1_sb, 1.0, 1.0,
                                        mybir.AluOpType.min,
                                        mybir.AluOpType.subtract)
            t2y_ps = psB.tile([32, tz], F32, tag="small")
            mm(t2y_ps, smat_s, t2_sb, start=True, stop=True)
            w_sb = work.tile([32, tz], F32, tag="w")
            nc.vector.tensor_mul(w_sb, t2_sb[0:32, :], t2y_ps)
            return m_sb, v_sb, w_sb, tz

        def stage2a_rest(wstate, o0=0, osz=None, drain=False):
            """qk head-sum, softmax partial sums (w precomputed)."""
            m_sb, v_sb, w_all, tz = wstate
            if osz is None:
                osz = tz
            so = slice(o0, o0 + osz)
            tz = osz
            w_sb = w_all[:, so]

            qk_ps = psB.tile([32, tz], F32, tag="small")
            if m8:
                mm(qk_ps, amat_s, m_sb[:, :, so], start=True, stop=True,
                   perf_mode=DR)
            else:
                mm(qk_ps, amat_s[:, 0:32], m_sb[:, 0, so], start=True,
                   stop=False)
                mm(qk_ps, amat_s[:, 32:64], m_sb[:, 1, so], start=False,
                   stop=True)
            lg_sb = work.tile([32, tz], F32, tag="lg")
            nc.vector.tensor_mul(lg_sb, qk_ps, w_sb)
            escale = SIGMA if m8 else SIGMA / (ALPHA * ALPHA)
            e_sb = work.tile([32, tz], F32R, tag="e")
            nc.scalar.activation(e_sb, lg_sb, AF.Exp, bias=0.0, scale=escale)
            s1_ps = psB.tile([8, tz], F32, tag="small")
            mm(s1_ps, cmat_s, e_sb, start=True, stop=True)
            ew_sb = work.tile([32, tz], F32R, tag="ew")
            if ew_dve and not drain:
                nc.vector.tensor_mul(ew_sb, e_sb, w_sb)
            else:
                nc.gpsimd.tensor_mul(ew_sb, e_sb, w_sb)
            s2_ps = psB.tile([8, tz], F32, tag="small")
            mm(s2_ps, cmat_s, ew_sb, start=True, stop=True)
            return s1_ps, s2_ps, v_sb, tz, o0

        def stage2a(state, o0=0, osz=None, drain=False):
            """Head-sum of q*k, grid-sample weight w, softmax partial sums.
            o0/osz optionally restrict to a token sub-range of the tile
            (used to pipeline the final tile's drain)."""
            m_sb, v_sb, off_ps, tz = state
            if osz is None:
                osz = tz
            so = slice(o0, o0 + osz)
            tz = osz

            qk_ps = psB.tile([32, tz], F32, tag="small")
            if m8:
                mm(qk_ps, amat_s, m_sb[:, :, so], start=True, stop=True,
                   perf_mode=DR)
            else:
                mm(qk_ps, amat_s[:, 0:32], m_sb[:, 0, so], start=True,
                   stop=False)
                mm(qk_ps, amat_s[:, 32:64], m_sb[:, 1, so], start=False,
                   stop=True)

            # w = relu(1-|sp_x-.5|)*relu(1-|sp_y-.5|); computed sign-flipped
            # as t2n = min(t1,1)-1 = -relu(1-t1) on GPSIMD (one tensor_scalar)
            # -- the two negative factors cancel in the product. The y half
            # is moved to partitions 0-31 with a PE row-select matmul since
            # DVE can't pair operands at different base partitions.
            t1_sb = work.tile([64, tz], F32, tag="t1")
            nc.scalar.activation(t1_sb, off_ps[:, so], AF.Abs, bias=bwof_s,
                                 scale=1.0)
            t2_sb = work.tile([64, tz], F32R, tag="t2")
            if t2_act:
                nc.scalar.activation(t2_sb, t1_sb, AF.Relu, bias=1.0,
                                     scale=-1.0)
            else:
                nc.gpsimd.tensor_scalar(t2_sb, t1_sb, 1.0, 1.0,
                                        mybir.AluOpType.min,
                                        mybir.AluOpType.subtract)
            t2y_ps = psB.tile([32, tz], F32, tag="small")
            mm(t2y_ps, smat_s, t2_sb, start=True, stop=True)
            w_sb = work.tile([32, tz], F32, tag="w")
            nc.vector.tensor_mul(w_sb, t2_sb[0:32, :], t2y_ps)

            # softmax over K: e = exp(qk*w/(256*sqrt(D))) -- the 1/256
            # unscales the 16x on each of q and k; the qk head-sum is read
            # straight from PSUM; e*w runs on the otherwise idle GPSIMD
            lg_sb = work.tile([32, tz], F32, tag="lg")
            nc.vector.tensor_mul(lg_sb, qk_ps, w_sb)
            escale = SIGMA if m8 else SIGMA / (ALPHA * ALPHA)
            if fuse_s:
                # e and e*w stacked in one [64,T] tile -> ONE [64->16]
                # matmul yields both softmax partial sums
                eew_sb = work.tile([64, tz], F32R, tag="eew")
                nc.scalar.activation(eew_sb[0:32, :], lg_sb, AF.Exp,
                                     bias=0.0, scale=escale)
                nc.vector.tensor_mul(eew_sb[32:64, :], eew_sb[0:32, :], w_sb)
                s12_ps = psB.tile([16, tz], F32, tag="small")
                mm(s12_ps, cmat2_s, eew_sb, start=True, stop=True)
                s1_ps, s2_ps = s12_ps[0:8, :], s12_ps[8:16, :]
            else:
                e_sb = work.tile([32, tz], F32R, tag="e")
                nc.scalar.activation(e_sb, lg_sb, AF.Exp, bias=0.0,
                                     scale=escale)
                # s1 fires immediately after exp so recip overlaps the e*w
                # product feeding s2
                s1_ps = psB.tile([8, tz], F32, tag="small")
                mm(s1_ps, cmat_s, e_sb, start=True, stop=True)
                ew_sb = work.tile([32, tz], F32R, tag="ew")
                if ew_dve and not drain:
                    nc.vector.tensor_mul(ew_sb, e_sb, w_sb)
                else:
                    nc.gpsimd.tensor_mul(ew_sb, e_sb, w_sb)
                s2_ps = psB.tile([8, tz], F32, tag="small")
                mm(s2_ps, cmat_s, ew_sb, start=True, stop=True)
            return s1_ps, s2_ps, v_sb, tz, o0

        def stage2b(state, g0, drain=False):
            """Softmax normalization, ov = v*wv, out-projection, store."""
            s1_ps, s2_ps, v_sb, tz, o0 = state
            so = slice(o0, o0 + tz)
            r1_sb = work.tile([8, tz], F32, tag="r1")
            nc.vector.reciprocal(r1_sb, s1_ps)
            wv_sb = work.tile([8, tz], F32R, tag="wvv")
            nc.vector.tensor_mul(wv_sb, s2_ps, r1_sb)

            # ov = v * wv (broadcast head->channels via matmul; bmat carries
            # 1/16 to unscale the 16x on v)
            ov_sb = work.tile([128, 2, tz], F32R, tag="ov")
            for mc in range(2):
                wvx_ps = psB.tile([128, tz], F32, tag="small")
                mm(wvx_ps, bmat_s[:, mc * 128:(mc + 1) * 128], wv_sb,
                   start=True, stop=True)
                nc.vector.tensor_mul(ov_sb[:, mc, :], v_sb[:, mc, so],
                                     wvx_ps)

            # outT = Wout.T @ ov, channel-major [256, T] (stationary = Wout
            # chunks: 4 LD_WEIGHTS instead of 8); host transposes back
            o_sb = work.tile([128, 2, tz], F32, tag="osb")
            for oc in range(2):
                o_ps = psB.tile([128, tz], F32, tag="small")
                oc128 = slice(oc * 128, (oc + 1) * 128)
                mm(o_ps, wo_s[:, 0, oc128], ov_sb[:, 0, :], start=True,
                   stop=False)
                mm(o_ps, wo_s[:, 1, oc128], ov_sb[:, 1, :], start=False,
                   stop=not with_bias)
                if with_bias:
                    mm(o_ps, bor_s[0:1, oc128], ones_s[0:1, :tz],
                       start=False, stop=True)
                if oc == 0 and not drain:
                    nc.vector.tensor_copy(o_sb[:, oc, :], o_ps)
                else:
                    nc.scalar.copy(o_sb[:, oc, :], o_ps)
            oeng = (nc.gpsimd if out_pool
                    else nc.scalar if store_act else nc.sync)
            if split_store:
                for oc in range(2):
                    oeng.dma_start(out=out[:, oc, g0 + o0:g0 + o0 + tz],
                                   in_=o_sb[:, oc, :])
            else:
                oeng.dma_start(out=out[:, :, g0 + o0:g0 + o0 + tz], in_=o_sb)

        # 3-deep software pipeline: per iteration emit tile i's matmul-heavy
        # stage1, then tile i-2's output tail (stage2b), then tile i-1's
        # softmax chain (stage2a) — PE stays dense while ACT/DVE chains of
        # earlier tiles drain. stage2b(i-2) must precede stage2a(i-1) so the
        # s1/s2 PSUM slots recycle in trace order.
        assert nsub in (1, 2)
        units = []
        for lt in range(nload):
            for lo in range(0, tload, TCOMP):
                if lt >= nload - last_split:
                    units.append((lt, lo, TCOMP // 2))
                    units.append((lt, lo + TCOMP // 2, TCOMP // 2))
                else:
                    units.append((lt, lo, TCOMP))
        p1 = p2 = None  # (state, g0) for stage2a / stage2b
        # first input tile before the weights so the PE can start ASAP;
        # weights ordered by first use
        wv_s = load1("wv", (128, 2, 2, 256), F8)
        ld = load_tile(0, spread=spread0)
        wq_s = load1("wq", (128, 2, 2, 256), F8)
        wp_s = load1("wp", (128, 2, 2, 256), F8)
        wk_s = load1("wk", (128, 2, 2, 256), F8)
        if early_load and nload > 1:
            if spread1:
                _sv = nc.sync
                nc.sync = nc.gpsimd
                ld1 = load_tile(1)
                nc.sync = _sv
            else:
                ld1 = load_tile(1)
        else:
            ld1 = None
        wo1_s = load1("wo1", (128, 2, 2, 512), F8)
        bo1_s = load1("bo1", (128, 4))
        amat_s = load1("amat", (128, 2, 32), F8)
        wo2_s = load1("wo2", (128, 4, 64))
        pmat_s = load1("pmat", (2, 64))
        bwof_s = load1("bwof", (64, 1))
        smat_s = load1("smat", (64, 32))
        cmat_s = load1("cmat", (32, 8))
        cmat2_s = load1("cmat2", (64, 16)) if fuse_s else None
        bmat_s = load1("bmat", (8, 256))
        wo_s = load1("wo", (128, 2, 256))
        if with_bias:
            bqp_s = load1("bqp", (1, 256))
            bkp_s = load1("bkp", (1, 256))
            bvr_s = load1("bvr", (1, 256))
            bor_s = load1("bor", (1, 256))
            ones_s = load1("ones", (1, 512))
        lds = {0: ld}
        if early_load and nload > 1:
            lds[1] = ld1
        for pf in range(1, min(prefetch, nload)):
            if pf not in lds:
                lds[pf] = load_tile(pf)
        for ui, (lt, lo, tz) in enumerate(units):
            nxt = ui + prefetch
            if nxt < len(units) and units[nxt][0] not in lds:
                lds[units[nxt][0]] = load_tile(units[nxt][0])
            ld = lds[lt]
            vl = True if (len(units) - ui) <= vlast else None
            hf = (len(units) - ui) <= hflast
            if order == 0:
                state = stage1(ld, lo, tz, vlate=vl, hid_first=hf)
                if ws:
                    state = stage2a_w(state)
                if p2 is not None:
                    stage2b(*p2)
                    p2 = None
                if p1 is not None:
                    st2, g0p = p1
                    p2 = ((stage2a_rest(st2) if ws else stage2a(st2)), g0p)
            elif order == 1:
                if p2 is not None:
                    stage2b(*p2)
                    p2 = None
                state = stage1(ld, lo, tz)
                if p1 is not None:
                    st2, g0p = p1
                    p2 = (stage2a(st2), g0p)
            elif order == 2:
                if p2 is not None:
                    stage2b(*p2)
                    p2 = None
                if p1 is not None:
                    st2, g0p = p1
                    p2 = (stage2a(st2), g0p)
                state = stage1(ld, lo, tz)
            else:  # order == 3: 2-deep, 2a(i) right in iteration i
                state = stage1(ld, lo, tz)
                if p2 is not None:
                    stage2b(*p2)
                    p2 = None
                p2 = (stage2a((state, )[0]), lt * tload + lo)
            p1 = (state, lt * tload + lo)
        if p2 is not None:
            stage2b(*p2)
        if order != 3:
            st2, g0p = p1
            ftz = st2[3]
            nchunk = max(1, tail_split)
            csz = ftz // nchunk
            dr = drl > 0
            s2a = stage2a_rest if ws else stage2a
            parts = [s2a(st2, o0=ci * csz, osz=csz, drain=dr)
                     for ci in range(nchunk)]
            for part in parts:
                stage2b(part, g0p, drain=dr)

    nc.compile()
    return nc


def _consts():
    amat = np.zeros((128, 64), np.float32)
    for mc in range(2):
        for d in range(128):
            h = mc * 4 + d // 32
            for k in range(KP):
                amat[d, mc * 32 + h * KP + k] = 1.0
    cmat = np.zeros((32, 8), np.float32)
    for j in range(32):
        cmat[j, j // KP] = 1.0
    cmat2 = np.zeros((64, 16), np.float32)
    for j in range(32):
        cmat2[j, j // KP] = 1.0
        cmat2[32 + j, 8 + j // KP] = 1.0
    bmat = np.zeros((8, 256), np.float32)
    for mc in range(2):
        for c in range(128):
            bmat[mc * 4 + c // 32, mc * 128 + c] = 1.0 / ALPHA
    pmat = np.zeros((2, 64), np.float32)
    for r in range(64):
        pmat[r // 32, r] = 1.0
    smat = np.zeros((64, 32), np.float32)
    for j in range(32):
        smat[32 + j, j] = 1.0
    return amat, cmat, cmat2, bmat, pmat, smat


def _wsplit(w):
    # [256, O] -> [128, 2, O]  (row kc*128+p  ->  [p, kc, :])
    o = w.shape[1]
    return np.ascontiguousarray(w.reshape(2, 128, o).transpose(1, 0, 2))


def _xsplit(x):
    # [T, 256] token-major -> [128, 2, T] channel-major chunks
    t = x.shape[0]
    return np.ascontiguousarray(x.T.reshape(2, 128, t).transpose(1, 0, 2))


def _split8(a):
    # [128, 2, O] f32 -> [128, 2, 2, O] fp8 (dim2 = hi, lo of the value)
    hi = a.astype(E4)
    lo = (a - hi.astype(np.float32)).astype(E4)
    return np.ascontiguousarray(np.stack([hi, lo], axis=2))


def _host_maps(inputs, toks, ncores):
    f32 = lambda v: np.asarray(v, dtype=np.float32)
    query = f32(inputs["query"]).reshape(-1, C)
    key = f32(inputs["key"]).reshape(-1, C)
    value = f32(inputs["value"]).reshape(-1, C)
    pos = f32(inputs["pos_embed"]).reshape(-1, C)
    refp = f32(inputs["reference_points"]).reshape(-1, 2)

    # permute Wo2 columns (h,k,c) -> (c,h,k)
    perm = [h * (KP * 2) + k * 2 + c for c in range(2) for h in range(H)
            for k in range(KP)]
    wo2p = f32(inputs["Wo2"])[:, perm]
    bo2p = f32(inputs["bo2"])[perm]

    amat, cmat, cmat2, bmat, pmat, smat = _consts()
    bqp = f32(inputs["bq"]) + f32(inputs["bpos"])
    bkp = f32(inputs["bk"]) + f32(inputs["bpos"])
    bv = f32(inputs["bv"])
    bout = f32(inputs["bout"])
    with_bias = any(np.any(b != 0) for b in (bqp, bkp, bv, bout))

    # Wo2 operand (hidden) carries 16x -> divide Wo2 by 16 to compensate
    wo2r = np.ascontiguousarray(
        (wo2p / ALPHA).reshape(4, 128, 64).transpose(1, 0, 2))
    shared = {
        "wq": _split8(_wsplit(ALPHA * f32(inputs["Wq"]))),
        "wk": _split8(_wsplit(ALPHA * f32(inputs["Wk"]))),
        "wv": _split8(_wsplit(ALPHA * f32(inputs["Wv"]))),
        "wp": _split8(_wsplit(ALPHA * f32(inputs["Wpos"]))),
        "wo1": _split8(_wsplit(ALPHA * f32(inputs["Wo1"]))),
        "wo2": wo2r,
        "wo": _wsplit(f32(inputs["Wout"])),
        "bo1": np.ascontiguousarray(
            (ALPHA * f32(inputs["bo1"])).reshape(4, 128).T),
        "bwof": np.ascontiguousarray((bo2p - 0.5).reshape(64, 1)),
        "smat": smat,
        "amat": np.ascontiguousarray(amat.reshape(128, 2, 32)).astype(E4),
        "cmat": cmat, "cmat2": cmat2, "bmat": bmat, "pmat": pmat,
    }
    if with_bias:
        shared["ones"] = np.ones((1, 512), np.float32)
        shared["bqp"] = ALPHA * bqp.reshape(1, 256)
        shared["bkp"] = ALPHA * bkp.reshape(1, 256)
        shared["bvr"] = ALPHA * bv.reshape(1, 256)
        shared["bor"] = bout.reshape(1, 256)

    in_maps = []
    for cid in range(ncores):
        sl = slice(cid * toks, (cid + 1) * toks)
        m = dict(shared)
        m["xq"] = _split8(_xsplit(query[sl]))
        m["xk"] = _split8(_xsplit(key[sl]))
        m["xv"] = _split8(_xsplit(value[sl]))
        m["xp"] = _split8(_xsplit(pos[sl]))
        m["ref"] = np.ascontiguousarray(refp[sl].T)
        in_maps.append(m)
    return in_maps, with_bias


_NC_CACHE = {}


def kernel(**inputs):
    from concourse.bass_utils import run_bass_kernel_spmd

    in_maps, with_bias = _host_maps(inputs, TOKS, NCORES)
    ck = ("full", with_bias)
    if ck not in _NC_CACHE:
        _NC_CACHE[ck] = _build(toks=TOKS, tload=TLOAD, with_bias=with_bias)
    nc = _NC_CACHE[ck]
    res = run_bass_kernel_spmd(nc, in_maps, core_ids=list(range(NCORES)))
    # out is channel-major [128, 2, toks]: out[t, kc*128+p] = o[p, kc, t]
    outs = [np.asarray(r["out"]).transpose(2, 1, 0).reshape(TOKS, C)
            for r in res.results]
    full = np.concatenate(outs, axis=0).reshape(N, L, C)
    return np.ascontiguousarray(full.astype(np.float32))
